# revision 38
# baseline (speedup 1.0000x reference)
"""Causal self-attention (RoPE on k/v) TRN2 Bass kernel — bf16 pipeline.

Sharding: core i handles batch b = i//2 and 8 heads hs = 8*(i%2).
Each core computes the qkv projection for its (batch, head-group), RoPE
on k and v, causal attention, and a partial output projection y^T with
its W_proj row-block.  Host sums the two partials per batch, adds
b_proj.

Structure (single TileContext, minimal phase gaps):
  A: qkv projection.  x^T resident in SBUF (bf16, streamed in 4 T-chunks
     so compute starts early).  q^T and rope(k)^T written to a resident
     SBUF tile qkT_sb [128, 16, T] (bf16) — no DRAM round-trip.  rope(v)
     streamed to DRAM (vr, bf16) in natural [T, d] layout.
  B: attention per head.  Scores/PV free dims clipped to the causal
     prefix at 128-key granularity (exact causal work).  Softmax 1/l via
     exp(-ln l) on the Scalar engine (avoids the 3.3us serial DVE
     reciprocal).  Attention out written to resident SBUF o_sb (bf16).
     W_proj prefetched at the start of B.
  C: out projection from o_sb, partial y^T written bf16.

Head-dim permutation (even dims first) turns RoPE's interleaved
even/odd pairs into contiguous 64-row/col halves; W_attn columns and
W_proj rows are permuted correspondingly on host, which leaves the
attention math invariant.

All matmuls in bf16 (1 PE cycle/row at any moving size, enabling the
causal clipping; fp32 PSUM accumulate).
"""
import sys

sys.path.insert(0, "/opt/trn_rl_repo")

import ml_dtypes
import numpy as np

import concourse.bass as bass  # noqa: F401
import concourse.mybir as mybir
import concourse.tile as tile
from concourse import bacc
from concourse.bass_utils import run_bass_kernel_spmd

B, T, C, H = 4, 2048, 2048, 16
HD = 128
HC = 8  # heads per core
NCORES = 8
F32 = mybir.dt.float32
BF16 = mybir.dt.bfloat16
BF16_NP = ml_dtypes.bfloat16
SCALE = float(1.0 / np.sqrt(HD))

_CACHE = {}


def _build_nc():
    nc = bacc.Bacc(num_devices=NCORES)

    xT = nc.dram_tensor("xT", [C, T], BF16, kind="ExternalInput")
    wqk = nc.dram_tensor("wqk", [C, 2048], BF16, kind="ExternalInput")
    bqk = nc.dram_tensor("bqk", [128, 16], F32, kind="ExternalInput")
    wv = nc.dram_tensor("wv", [C, 1024], BF16, kind="ExternalInput")
    bv = nc.dram_tensor("bv", [128, 1024], F32, kind="ExternalInput")
    wp = nc.dram_tensor("wp", [1024, C], BF16, kind="ExternalInput")
    rtab_u = nc.dram_tensor("rtab_u", [128, T], BF16, kind="ExternalInput")
    rtab_v = nc.dram_tensor("rtab_v", [128, T], BF16, kind="ExternalInput")
    cos4 = nc.dram_tensor("cos4", [T, 256], BF16, kind="ExternalInput")
    sin4 = nc.dram_tensor("sin4", [T, 256], BF16, kind="ExternalInput")
    # lneg[p, i] = -1e9 where i < p (causal mask addend), ident = I
    lneg = nc.dram_tensor("lneg", [128, 128], BF16, kind="ExternalInput")
    ident = nc.dram_tensor("ident", [128, 128], BF16, kind="ExternalInput")
    yT = nc.dram_tensor("yT", [C, T], BF16, kind="ExternalOutput")

    vr_d = nc.dram_tensor("vr_d", [T, 1024], BF16)

    with tile.TileContext(nc) as tc:
        with tc.tile_pool(name="tabs", bufs=1) as tabp, \
             tc.tile_pool(name="qksb", bufs=1) as qkp, \
             tc.tile_pool(name="hv", bufs=2) as hvp:
            # ---- persistent: resident qk + small tables ----
            qkT_sb = qkp.tile([128, 16, T], BF16)
            bqk_t = tabp.tile([128, 16], F32)
            lneg_t = tabp.tile([128, 128], BF16)
            ident_t = tabp.tile([128, 128], BF16)
            ones_f = tabp.tile([128, 128], F32)
            ones_b = tabp.tile([128, 128], BF16)
            vr_r = vr_d.rearrange("(jt p) d -> p jt d", p=128)

            def load_vh_kc(t_, hh, kc):
                nc.sync.dma_start(
                    t_[:, bass.ts(kc, 4), :],
                    vr_r[:, bass.ts(kc, 4), hh * 128:(hh + 1) * 128])

            def load_vh(hh):
                t_ = hvp.tile([128, 16, 128], BF16, tag="vh")
                for kc in range(4):
                    load_vh_kc(t_, hh, kc)
                return t_

            # ------------- Phase A: qkv projection + rope -------------
            with tc.tile_pool(name="atab", bufs=1) as atabp, \
                 tc.tile_pool(name="xt", bufs=1) as xtp, \
                 tc.tile_pool(name="wvf", bufs=1) as wvp, \
                 tc.tile_pool(name="wblk", bufs=2) as wbp, \
                 tc.tile_pool(name="cs", bufs=2) as csp, \
                 tc.tile_pool(name="ktmp", bufs=3) as ktp, \
                 tc.tile_pool(name="vtmp", bufs=2) as vtp, \
                 tc.tile_pool(name="vro", bufs=2) as vrop, \
                 tc.tile_pool(name="psA", bufs=5, space="PSUM") as psp:
                xt = xtp.tile([128, 16, T], BF16)
                xT_r = xT.rearrange("(a p) t -> p a t", p=128)
                # first T-chunks of x and the first weight block lead the
                # DMA issue order so the first matmul chains start early
                for sub in range(2):
                    nc.sync.dma_start(
                        xt[:, bass.ts(sub, 8), 0:512],
                        xT_r[:, bass.ts(sub, 8), 0:512])
                wqk_r = wqk.rearrange("(a p) j -> p a j", p=128)
                wblk0 = wbp.tile([128, 16, 128], BF16, tag="wblk")
                nc.sync.dma_start(wblk0[:], wqk_r[:, :, 0:128])
                for sub in range(2):
                    nc.sync.dma_start(
                        xt[:, bass.ts(sub, 8), 512:1024],
                        xT_r[:, bass.ts(sub, 8), 512:1024])
                nc.sync.dma_start(bqk_t[:], bqk[:, :])
                for tc4 in range(2, 4):
                    nc.sync.dma_start(
                        xt[:, :, bass.ts(tc4, 512)],
                        xT_r[:, :, bass.ts(tc4, 512)])
                ut = atabp.tile([128, T], BF16)
                vt_tab = atabp.tile([128, T], BF16)
                wvf = wvp.tile([128, 16, 1024], BF16)
                wv_r = wv.rearrange("(a p) d -> p a d", p=128)
                bv_t = atabp.tile([128, 1024], F32)
                nc.vector.memset(ones_f[:], 1.0)
                nc.vector.tensor_copy(ones_b[:], ones_f[:])
                cos4_r = cos4.rearrange("(a p) i -> p a i", p=128)
                sin4_r = sin4.rearrange("(a p) i -> p a i", p=128)

                def late_dmas(jt):
                    # deferred into the loop: lets the xt chunks win the
                    # HBM bandwidth race during startup
                    if jt == 1:
                        nc.sync.dma_start(ut[:], rtab_u[:, :])
                        nc.sync.dma_start(vt_tab[:], rtab_v[:, :])
                        nc.sync.dma_start(lneg_t[:], lneg[:, :])
                        nc.sync.dma_start(ident_t[:], ident[:, :])
                    elif jt == 2:
                        nc.sync.dma_start(bv_t[:], bv[:, :])
                        for wc in range(2):
                            nc.sync.dma_start(
                                wvf[:, bass.ts(wc, 4), :],
                                wv_r[:, bass.ts(wc, 4), :])
                    elif jt == 3:
                        for wc in range(2, 4):
                            nc.sync.dma_start(
                                wvf[:, bass.ts(wc, 4), :],
                                wv_r[:, bass.ts(wc, 4), :])

                # ---- A-qk: q^T and rope(k)^T into qkT_sb ----
                wblk = wblk0
                for jt in range(16):
                    late_dmas(jt)
                    if jt + 1 < 16:
                        wblk_n = wbp.tile([128, 16, 128], BF16, tag="wblk")
                        nc.sync.dma_start(
                            wblk_n[:],
                            wqk_r[:, :, (jt + 1) * 128:(jt + 2) * 128])
                    for tb in range(4):
                        ts = bass.ts(tb, 512)
                        ps = psp.tile([128, 512], F32, tag="ps")
                        for c in range(16):
                            nc.tensor.matmul(
                                ps[:], wblk[:, c], xt[:, c, ts],
                                start=(c == 0), stop=(c == 15))
                        if jt < 8:
                            # q: bias add on the Scalar engine, bf16 out
                            nc.scalar.activation(
                                qkT_sb[:, jt, ts], ps[:],
                                mybir.ActivationFunctionType.Identity,
                                bias=bqk_t[:, jt:jt + 1])
                        else:
                            # k: bias on Scalar, rope on DVE (swap via DMA,
                            # in-place multiplies)
                            kt = ktp.tile([128, 512], BF16, tag="kt")
                            nc.scalar.activation(
                                kt[:], ps[:],
                                mybir.ActivationFunctionType.Identity,
                                bias=bqk_t[:, jt:jt + 1])
                            kts = ktp.tile([128, 512], BF16, tag="kts")
                            nc.sync.dma_start(kts[0:64, :], kt[64:128, :])
                            nc.sync.dma_start(kts[64:128, :], kt[0:64, :])
                            nc.vector.tensor_mul(kt[:], kt[:], ut[:, ts])
                            nc.vector.tensor_mul(
                                kts[:], kts[:], vt_tab[:, ts])
                            nc.vector.tensor_add(
                                qkT_sb[:, jt, ts], kt[:], kts[:])
                    wblk = wblk_n if jt + 1 < 16 else None

                # ---- A-v: rope(v) natural layout -> DRAM ----
                vh0 = hvp.tile([128, 16, 128], BF16, tag="vh")
                vh1 = hvp.tile([128, 16, 128], BF16, tag="vh")
                for tt in range(16):
                    c4t = csp.tile([128, 256], BF16, tag="c4")
                    nc.sync.dma_start(c4t[:], cos4_r[:, tt, :])
                    s4t = csp.tile([128, 256], BF16, tag="s4")
                    nc.sync.dma_start(s4t[:], sin4_r[:, tt, :])
                    c43 = c4t[:].rearrange("p (h d) -> p h d", h=4)
                    s43 = s4t[:].rearrange("p (h d) -> p h d", h=4)
                    for db in range(2):
                        ds = bass.ts(db, 512)
                        ps = psp.tile([128, 512], F32, tag="ps")
                        for c in range(16):
                            nc.tensor.matmul(
                                ps[:], xt[:, c, bass.ts(tt, 128)],
                                wvf[:, c, ds], start=(c == 0), stop=(c == 15))
                        vb = vtp.tile([128, 512], BF16, tag="vb")
                        nc.vector.tensor_add(vb[:], ps[:], bv_t[:, ds])
                        v3 = vb[:].rearrange("p (h d) -> p h d", h=4)
                        me = vtp.tile([128, 4, 64], BF16, tag="me")
                        mo = vtp.tile([128, 4, 64], BF16, tag="mo")
                        vro = vrop.tile([128, 512], BF16, tag="vro")
                        vr3 = vro[:].rearrange("p (h d) -> p h d", h=4)
                        nc.vector.tensor_mul(
                            me[:], v3[:, :, 0:64], c43[:, :, 0:64])
                        nc.vector.tensor_mul(
                            mo[:], v3[:, :, 64:128], s43[:, :, 0:64])
                        nc.vector.tensor_sub(
                            vr3[:, :, 0:64], me[:], mo[:])
                        nc.vector.tensor_mul(
                            me[:], v3[:, :, 0:64], s43[:, :, 0:64])
                        nc.vector.tensor_mul(
                            mo[:], v3[:, :, 64:128], c43[:, :, 0:64])
                        nc.vector.tensor_add(
                            vr3[:, :, 64:128], me[:], mo[:])
                        nc.sync.dma_start(
                            vr_d[bass.ts(tt, 128), ds], vro[:])
                    if tt % 4 == 3:
                        # prefetch first two heads' v as stores complete
                        load_vh_kc(vh0, 0, tt // 4)
                        load_vh_kc(vh1, 1, tt // 4)

            # ------------- Phases B + C -------------
            with tc.tile_pool(name="osb", bufs=1) as osbp, \
                 tc.tile_pool(name="wpb", bufs=1) as wpp:
                o_sb = osbp.tile([128, HC, T], BF16)
                wps = wpp.tile([128, 8, C], BF16)
                # prefetch W_proj under phase-B compute (split for parallelism)
                wp_r = wp.rearrange("(ht p) c -> p ht c", p=128)
                for ht in range(8):
                    nc.sync.dma_start(wps[:, ht, :], wp_r[:, ht, :])

                with tc.tile_pool(name="pt", bufs=6) as ptp, \
                     tc.tile_pool(name="bsc", bufs=3) as bscp, \
                     tc.tile_pool(name="psS", bufs=2, space="PSUM") as psp, \
                     tc.tile_pool(name="lps", bufs=2, space="PSUM") as lpsp, \
                     tc.tile_pool(name="ops", bufs=2, space="PSUM") as opsp:
                    vhs = [vh0, vh1]
                    for h in range(HC):
                        vh = vhs[h]
                        # last head ascending: C's tb=0 tiles unblock early
                        # and the final normalize latency hides under C
                        ib_order = (0, 1, 2, 3) if h == HC - 1 else (3, 2, 1, 0)
                        for ib in ib_order:
                            isl = bass.ts(ib, 512)
                            nj = 4 * ib + 4
                            l_ps = lpsp.tile([128, 512], F32, tag="l")
                            o_ps = opsp.tile([128, 512], F32, tag="o")
                            pts = [None] * nj
                            qss = [max(0, 128 * jt - 512 * ib)
                                   for jt in range(nj)]

                            def emit_s_pair(j0, h=h, ib=ib, qss=qss, pts=pts):
                                # two score tiles in one 2-bank PSUM tile,
                                # ONE exp instruction for both halves (the
                                # ~260ns fixed ACT overhead dominates the
                                # clipped exp tiles otherwise)
                                s2 = psp.tile([128, 2, 512], F32, tag="ps")
                                for half in range(2):
                                    jt = j0 + half
                                    qs = qss[jt]
                                    diag = jt >= 4 * ib
                                    nc.tensor.matmul(
                                        s2[:, half, qs:],
                                        qkT_sb[:, 8 + h, bass.ts(jt, 128)],
                                        qkT_sb[:, h,
                                               512 * ib + qs:512 * (ib + 1)],
                                        start=True, stop=not diag)
                                    if diag:
                                        # causal mask: add -1e9 above the
                                        # diagonal via identity-weights matmul
                                        nc.tensor.matmul(
                                            s2[:, half, qs:qs + 128],
                                            ident_t[:], lneg_t[:],
                                            start=False, stop=True)
                                qs0 = qss[j0]
                                pt2 = ptp.tile([128, 2, 512], BF16, tag="pt")
                                nc.scalar.activation(
                                    pt2[:, :, qs0:], s2[:, :, qs0:],
                                    mybir.ActivationFunctionType.Exp,
                                    scale=SCALE)
                                pts[j0] = pt2[:, 0]
                                pts[j0 + 1] = pt2[:, 1]

                            def consume4(j0, vh=vh, l_ps=l_ps, o_ps=o_ps,
                                         pts=pts, qss=qss, nj=nj):
                                # same-chain runs back-to-back: the PE
                                # streams chained same-bank matmuls at full
                                # rate; alternating banks costs ~150ns each
                                for jt in range(j0, j0 + 4):
                                    nc.tensor.matmul(
                                        l_ps[:, qss[jt]:], ones_b[:],
                                        pts[jt][:, qss[jt]:],
                                        start=(jt == 0), stop=(jt == nj - 1))
                                for jt in range(j0, j0 + 4):
                                    nc.tensor.matmul(
                                        o_ps[:, qss[jt]:], vh[:, jt],
                                        pts[jt][:, qss[jt]:],
                                        start=(jt == 0), stop=(jt == nj - 1))

                            # quad-wise software pipeline, 2-quad lookahead
                            for g in range(0, nj, 4):
                                emit_s_pair(g)
                                emit_s_pair(g + 2)
                                if g >= 8:
                                    consume4(g - 8)
                            for g in sorted({max(0, nj - 8), nj - 4}):
                                consume4(g)

                            # 1/l: bf16 reciprocal on DVE
                            l_sb = bscp.tile([128, 512], BF16, tag="lsb")
                            nc.vector.tensor_copy(l_sb[:], l_ps[:])
                            r_sb = bscp.tile([128, 512], BF16, tag="r")
                            with nc.allow_low_precision(
                                    "softmax denom recip in bf16"):
                                nc.vector.reciprocal(r_sb[:], l_sb[:])
                            nc.vector.tensor_mul(
                                o_sb[:, h, isl], o_ps[:], r_sb[:])
                        if h + 2 < HC:
                            vhs.append(load_vh(h + 2))
                        else:
                            vhs.append(None)

                # ------------- Phase C: out projection -------------
                with tc.tile_pool(name="yo", bufs=4) as yop, \
                     tc.tile_pool(name="psC", bufs=4, space="PSUM") as pscp:
                    for tb in range(4):
                        ts = bass.ts(tb, 512)
                        for ct in range(16):
                            ps = pscp.tile([128, 512], F32, tag="ps")
                            for ht in range(8):
                                nc.tensor.matmul(
                                    ps[:], wps[:, ht, bass.ts(ct, 128)],
                                    o_sb[:, ht, ts],
                                    start=(ht == 0), stop=(ht == 7))
                            yo = yop.tile([128, 512], BF16, tag="yo")
                            nc.scalar.activation(
                                yo[:], ps[:],
                                mybir.ActivationFunctionType.Copy)
                            nc.sync.dma_start(
                                yT[ct * 128:(ct + 1) * 128, ts], yo[:])

    nc.compile()
    return nc


def _prep_inputs(x, freqs_cos, freqs_sin, W_attn, b_attn, W_proj):
    """Host-side sharding / layout prep.  Returns list of 8 in_maps."""
    perm = np.concatenate([np.arange(0, HD, 2), np.arange(1, HD, 2)])

    cosT = np.ascontiguousarray(freqs_cos.T)  # [64, T]
    sinT = np.ascontiguousarray(freqs_sin.T)
    rtab_u = np.concatenate([cosT, cosT], axis=0).astype(BF16_NP)
    rtab_v = np.concatenate([-sinT, sinT], axis=0).astype(BF16_NP)
    cos4 = np.tile(freqs_cos, (1, 4)).astype(BF16_NP)  # [T, 256]
    sin4 = np.tile(freqs_sin, (1, 4)).astype(BF16_NP)

    jj = np.arange(128)[:, None]
    ii = np.arange(128)[None, :]
    lneg = np.where(ii < jj, -1e9, 0.0).astype(BF16_NP)  # [128, 128]
    ident = np.eye(128).astype(BF16_NP)

    in_maps = []
    for core in range(NCORES):
        b = core // 2
        hs = HC * (core % 2)
        cols = np.concatenate(
            [g * HD + perm for g in range(hs, hs + HC)])  # [1024]

        wqk = np.concatenate(
            [W_attn[:, cols], W_attn[:, C + cols]], axis=1)
        bqk_flat = np.concatenate([b_attn[cols], b_attn[C + cols]])
        bqk = np.ascontiguousarray(
            bqk_flat.reshape(16, 128).T)  # [128, 16], bias[jt*128+p]
        wv = W_attn[:, 2 * C + cols]
        bv = np.broadcast_to(b_attn[2 * C + cols], (128, 1024))
        wp = W_proj[cols, :]

        in_maps.append({
            "xT": np.ascontiguousarray(x[b].T).astype(BF16_NP),
            "wqk": np.ascontiguousarray(wqk).astype(BF16_NP),
            "bqk": np.ascontiguousarray(bqk).astype(np.float32),
            "wv": np.ascontiguousarray(wv).astype(BF16_NP),
            "bv": np.ascontiguousarray(bv).astype(np.float32),
            "wp": np.ascontiguousarray(wp).astype(BF16_NP),
            "rtab_u": rtab_u,
            "rtab_v": rtab_v,
            "cos4": cos4,
            "sin4": sin4,
            "lneg": np.ascontiguousarray(lneg),
            "ident": np.ascontiguousarray(ident),
        })
    return in_maps


def kernel(x, freqs_cos, freqs_sin, mask, W_attn, b_attn, W_proj, b_proj,
           _return_results=False):
    x = np.asarray(x, dtype=np.float32)
    freqs_cos = np.asarray(freqs_cos, dtype=np.float32)
    freqs_sin = np.asarray(freqs_sin, dtype=np.float32)
    W_attn = np.asarray(W_attn, dtype=np.float32)
    b_attn = np.asarray(b_attn, dtype=np.float32)
    W_proj = np.asarray(W_proj, dtype=np.float32)
    b_proj = np.asarray(b_proj, dtype=np.float32)

    if "nc" not in _CACHE:
        _CACHE["nc"] = _build_nc()
    nc = _CACHE["nc"]

    in_maps = _prep_inputs(x, freqs_cos, freqs_sin, W_attn, b_attn, W_proj)
    res = run_bass_kernel_spmd(nc, in_maps, core_ids=list(range(NCORES)))

    out = np.empty((B, T, C), dtype=np.float32)
    for b in range(B):
        yt0 = res.results[2 * b]["yT"].astype(np.float32)
        yt1 = res.results[2 * b + 1]["yT"].astype(np.float32)
        out[b] = yt0.T + yt1.T + b_proj[None, :]
    if _return_results:
        return out, res
    return out


# revision 43
# speedup vs baseline: 1.0112x; 1.0112x over previous
"""Causal self-attention (RoPE on k/v) TRN2 Bass kernel — bf16 pipeline.

Sharding: core i handles batch b = i//2 and 8 heads hs = 8*(i%2).
Each core computes the qkv projection for its (batch, head-group), RoPE
on k and v, causal attention, and a partial output projection y^T with
its W_proj row-block.  Host sums the two partials per batch, adds
b_proj.

Structure (single TileContext, minimal phase gaps):
  A: qkv projection.  x^T resident in SBUF (bf16, streamed in 4 T-chunks
     so compute starts early).  q^T and rope(k)^T written to a resident
     SBUF tile qkT_sb [128, 16, T] (bf16) — no DRAM round-trip.  rope(v)
     streamed to DRAM (vr, bf16) in natural [T, d] layout.
  B: attention per head.  Scores/PV free dims clipped to the causal
     prefix at 128-key granularity (exact causal work).  Softmax 1/l via
     exp(-ln l) on the Scalar engine (avoids the 3.3us serial DVE
     reciprocal).  Attention out written to resident SBUF o_sb (bf16).
     W_proj prefetched at the start of B.
  C: out projection from o_sb, partial y^T written bf16.

Head-dim permutation (even dims first) turns RoPE's interleaved
even/odd pairs into contiguous 64-row/col halves; W_attn columns and
W_proj rows are permuted correspondingly on host, which leaves the
attention math invariant.

All matmuls in bf16 (1 PE cycle/row at any moving size, enabling the
causal clipping; fp32 PSUM accumulate).
"""
import sys

sys.path.insert(0, "/opt/trn_rl_repo")

import ml_dtypes
import numpy as np

import concourse.bass as bass  # noqa: F401
import concourse.mybir as mybir
import concourse.tile as tile
from concourse import bacc
from concourse.bass_utils import run_bass_kernel_spmd

B, T, C, H = 4, 2048, 2048, 16
HD = 128
HC = 8  # heads per core
NCORES = 8
F32 = mybir.dt.float32
BF16 = mybir.dt.bfloat16
BF16_NP = ml_dtypes.bfloat16
SCALE = float(1.0 / np.sqrt(HD))

_CACHE = {}


def _build_nc():
    nc = bacc.Bacc(num_devices=NCORES)

    xT = nc.dram_tensor("xT", [C, T], BF16, kind="ExternalInput")
    wqk = nc.dram_tensor("wqk", [C, 2048], BF16, kind="ExternalInput")
    bqk = nc.dram_tensor("bqk", [128, 16], F32, kind="ExternalInput")
    wv = nc.dram_tensor("wv", [C, 1024], BF16, kind="ExternalInput")
    bv = nc.dram_tensor("bv", [128, 1024], F32, kind="ExternalInput")
    wp = nc.dram_tensor("wp", [1024, C], BF16, kind="ExternalInput")
    rtab_u = nc.dram_tensor("rtab_u", [128, T], BF16, kind="ExternalInput")
    rtab_v = nc.dram_tensor("rtab_v", [128, T], BF16, kind="ExternalInput")
    cos4 = nc.dram_tensor("cos4", [T, 256], BF16, kind="ExternalInput")
    sin4 = nc.dram_tensor("sin4", [T, 256], BF16, kind="ExternalInput")
    # lneg[p, i] = -1e9 where i < p (causal mask addend), ident = I
    lneg = nc.dram_tensor("lneg", [128, 128], BF16, kind="ExternalInput")
    ident = nc.dram_tensor("ident", [128, 128], BF16, kind="ExternalInput")
    yT = nc.dram_tensor("yT", [C, T], BF16, kind="ExternalOutput")

    vr_d = nc.dram_tensor("vr_d", [T, 1024], BF16)

    with tile.TileContext(nc) as tc:
        with tc.tile_pool(name="tabs", bufs=1) as tabp, \
             tc.tile_pool(name="qksb", bufs=1) as qkp, \
             tc.tile_pool(name="hv", bufs=2) as hvp:
            # ---- persistent: resident qk + small tables ----
            qkT_sb = qkp.tile([128, 16, T], BF16)
            bqk_t = tabp.tile([128, 16], F32)
            lneg_t = tabp.tile([128, 128], BF16)
            ident_t = tabp.tile([128, 128], BF16)
            ones_f = tabp.tile([128, 128], F32)
            ones_b = tabp.tile([128, 128], BF16)
            vr_r = vr_d.rearrange("(jt p) d -> p jt d", p=128)

            def load_vh_kc(t_, hh, kc):
                nc.sync.dma_start(
                    t_[:, bass.ts(kc, 4), :],
                    vr_r[:, bass.ts(kc, 4), hh * 128:(hh + 1) * 128])

            def load_vh(hh):
                t_ = hvp.tile([128, 16, 128], BF16, tag="vh")
                for kc in range(4):
                    load_vh_kc(t_, hh, kc)
                return t_

            # ------------- Phase A: qkv projection + rope -------------
            with tc.tile_pool(name="atab", bufs=1) as atabp, \
                 tc.tile_pool(name="xt", bufs=1) as xtp, \
                 tc.tile_pool(name="wvf", bufs=1) as wvp, \
                 tc.tile_pool(name="wblk", bufs=2) as wbp, \
                 tc.tile_pool(name="cs", bufs=2) as csp, \
                 tc.tile_pool(name="ktmp", bufs=3) as ktp, \
                 tc.tile_pool(name="vtmp", bufs=2) as vtp, \
                 tc.tile_pool(name="vro", bufs=2) as vrop, \
                 tc.tile_pool(name="psA", bufs=6, space="PSUM") as psp:
                xt = xtp.tile([128, 16, T], BF16)
                xT_r = xT.rearrange("(a p) t -> p a t", p=128)
                # first T-chunks of x and the first weight block lead the
                # DMA issue order so the first matmul chains start early
                for sub in range(4):
                    nc.sync.dma_start(
                        xt[:, bass.ts(sub, 4), 0:512],
                        xT_r[:, bass.ts(sub, 4), 0:512])
                wqk_r = wqk.rearrange("(a p) j -> p a j", p=128)
                wblk0 = wbp.tile([128, 16, 128], BF16, tag="wblk")
                nc.sync.dma_start(wblk0[:], wqk_r[:, :, 0:128])
                for sub in range(4):
                    nc.sync.dma_start(
                        xt[:, bass.ts(sub, 4), 512:1024],
                        xT_r[:, bass.ts(sub, 4), 512:1024])
                nc.sync.dma_start(bqk_t[:], bqk[:, :])
                for tc4 in range(2, 4):
                    for sub in range(2):
                        nc.sync.dma_start(
                            xt[:, bass.ts(sub, 8), bass.ts(tc4, 512)],
                            xT_r[:, bass.ts(sub, 8), bass.ts(tc4, 512)])
                ut = atabp.tile([128, T], BF16)
                vt_tab = atabp.tile([128, T], BF16)
                wvf = wvp.tile([128, 16, 1024], BF16)
                wv_r = wv.rearrange("(a p) d -> p a d", p=128)
                bv_t = atabp.tile([128, 1024], F32)
                nc.vector.memset(ones_f[:], 1.0)
                nc.vector.tensor_copy(ones_b[:], ones_f[:])
                cos4_r = cos4.rearrange("(a p) i -> p a i", p=128)
                sin4_r = sin4.rearrange("(a p) i -> p a i", p=128)

                def late_dmas(jt):
                    # deferred into the loop: lets the xt chunks win the
                    # HBM bandwidth race during startup
                    if jt == 1:
                        nc.sync.dma_start(ut[:], rtab_u[:, :])
                        nc.sync.dma_start(vt_tab[:], rtab_v[:, :])
                        nc.sync.dma_start(lneg_t[:], lneg[:, :])
                        nc.sync.dma_start(ident_t[:], ident[:, :])
                    elif jt == 2:
                        nc.sync.dma_start(bv_t[:], bv[:, :])
                        for wc in range(2):
                            nc.sync.dma_start(
                                wvf[:, bass.ts(wc, 4), :],
                                wv_r[:, bass.ts(wc, 4), :])
                    elif jt == 3:
                        for wc in range(2, 4):
                            nc.sync.dma_start(
                                wvf[:, bass.ts(wc, 4), :],
                                wv_r[:, bass.ts(wc, 4), :])

                # ---- A-qk: q^T and rope(k)^T into qkT_sb ----
                wblk = wblk0
                for jt in range(16):
                    late_dmas(jt)
                    if jt + 1 < 16:
                        wblk_n = wbp.tile([128, 16, 128], BF16, tag="wblk")
                        nc.sync.dma_start(
                            wblk_n[:],
                            wqk_r[:, :, (jt + 1) * 128:(jt + 2) * 128])
                    for tb in range(4):
                        ts = bass.ts(tb, 512)
                        ps = psp.tile([128, 512], F32, tag="ps")
                        for c in range(16):
                            nc.tensor.matmul(
                                ps[:], wblk[:, c], xt[:, c, ts],
                                start=(c == 0), stop=(c == 15))
                        if jt < 8:
                            # q: bias add on the Scalar engine, bf16 out
                            nc.scalar.activation(
                                qkT_sb[:, jt, ts], ps[:],
                                mybir.ActivationFunctionType.Identity,
                                bias=bqk_t[:, jt:jt + 1])
                        else:
                            # k: bias on Scalar, rope on DVE (swap via DMA,
                            # in-place multiplies)
                            kt = ktp.tile([128, 512], BF16, tag="kt")
                            nc.scalar.activation(
                                kt[:], ps[:],
                                mybir.ActivationFunctionType.Identity,
                                bias=bqk_t[:, jt:jt + 1])
                            kts = ktp.tile([128, 512], BF16, tag="kts")
                            nc.sync.dma_start(kts[0:64, :], kt[64:128, :])
                            nc.sync.dma_start(kts[64:128, :], kt[0:64, :])
                            nc.vector.tensor_mul(kt[:], kt[:], ut[:, ts])
                            nc.vector.tensor_mul(
                                kts[:], kts[:], vt_tab[:, ts])
                            nc.vector.tensor_add(
                                qkT_sb[:, jt, ts], kt[:], kts[:])
                    wblk = wblk_n if jt + 1 < 16 else None

                # ---- A-v: rope(v) natural layout -> DRAM ----
                vh0 = hvp.tile([128, 16, 128], BF16, tag="vh")
                vh1 = hvp.tile([128, 16, 128], BF16, tag="vh")
                for tt in range(16):
                    c4t = csp.tile([128, 256], BF16, tag="c4")
                    nc.sync.dma_start(c4t[:], cos4_r[:, tt, :])
                    s4t = csp.tile([128, 256], BF16, tag="s4")
                    nc.sync.dma_start(s4t[:], sin4_r[:, tt, :])
                    c43 = c4t[:].rearrange("p (h d) -> p h d", h=4)
                    s43 = s4t[:].rearrange("p (h d) -> p h d", h=4)
                    for db in range(2):
                        ds = bass.ts(db, 512)
                        ps = psp.tile([128, 512], F32, tag="ps")
                        for c in range(16):
                            nc.tensor.matmul(
                                ps[:], xt[:, c, bass.ts(tt, 128)],
                                wvf[:, c, ds], start=(c == 0), stop=(c == 15))
                        vb = vtp.tile([128, 512], BF16, tag="vb")
                        nc.vector.tensor_add(vb[:], ps[:], bv_t[:, ds])
                        v3 = vb[:].rearrange("p (h d) -> p h d", h=4)
                        me = vtp.tile([128, 4, 64], BF16, tag="me")
                        mo = vtp.tile([128, 4, 64], BF16, tag="mo")
                        vro = vrop.tile([128, 512], BF16, tag="vro")
                        vr3 = vro[:].rearrange("p (h d) -> p h d", h=4)
                        nc.vector.tensor_mul(
                            me[:], v3[:, :, 0:64], c43[:, :, 0:64])
                        nc.vector.tensor_mul(
                            mo[:], v3[:, :, 64:128], s43[:, :, 0:64])
                        nc.vector.tensor_sub(
                            vr3[:, :, 0:64], me[:], mo[:])
                        nc.vector.tensor_mul(
                            me[:], v3[:, :, 0:64], s43[:, :, 0:64])
                        nc.vector.tensor_mul(
                            mo[:], v3[:, :, 64:128], c43[:, :, 0:64])
                        nc.vector.tensor_add(
                            vr3[:, :, 64:128], me[:], mo[:])
                        nc.sync.dma_start(
                            vr_d[bass.ts(tt, 128), ds], vro[:])
                    if tt % 4 == 3:
                        # prefetch first two heads' v as stores complete
                        load_vh_kc(vh0, 0, tt // 4)
                        load_vh_kc(vh1, 1, tt // 4)

            # ------------- Phases B + C -------------
            with tc.tile_pool(name="osb", bufs=1) as osbp, \
                 tc.tile_pool(name="wpb", bufs=1) as wpp:
                o_sb = osbp.tile([128, HC, T], BF16)
                wps = wpp.tile([128, 8, C], BF16)
                # prefetch W_proj under phase-B compute (split for parallelism)
                wp_r = wp.rearrange("(ht p) c -> p ht c", p=128)
                for ht in range(8):
                    nc.sync.dma_start(wps[:, ht, :], wp_r[:, ht, :])

                with tc.tile_pool(name="pt", bufs=12) as ptp, \
                     tc.tile_pool(name="bsc", bufs=3) as bscp, \
                     tc.tile_pool(name="psS", bufs=4, space="PSUM") as psp, \
                     tc.tile_pool(name="lps", bufs=2, space="PSUM") as lpsp, \
                     tc.tile_pool(name="ops", bufs=2, space="PSUM") as opsp:
                    vhs = [vh0, vh1]
                    for h in range(HC):
                        vh = vhs[h]
                        # last head ascending: C's tb=0 tiles unblock early
                        # and the final normalize latency hides under C
                        ib_order = (0, 1, 2, 3) if h == HC - 1 else (3, 2, 1, 0)
                        for ib in ib_order:
                            isl = bass.ts(ib, 512)
                            nj = 4 * ib + 4
                            l_ps = lpsp.tile([128, 512], F32, tag="l")
                            o_ps = opsp.tile([128, 512], F32, tag="o")
                            pts = [None] * nj
                            qss = [max(0, 128 * jt - 512 * ib)
                                   for jt in range(nj)]

                            def emit_s(jt, h=h, ib=ib, qss=qss, pts=pts):
                                qs = qss[jt]
                                diag = jt >= 4 * ib
                                s_ps = psp.tile([128, 512], F32, tag="ps")
                                nc.tensor.matmul(
                                    s_ps[:, qs:],
                                    qkT_sb[:, 8 + h, bass.ts(jt, 128)],
                                    qkT_sb[:, h, 512 * ib + qs:512 * (ib + 1)],
                                    start=True, stop=not diag)
                                if diag:
                                    # causal mask: add -1e9 above the
                                    # diagonal via identity-weights matmul
                                    nc.tensor.matmul(
                                        s_ps[:, qs:qs + 128],
                                        ident_t[:], lneg_t[:],
                                        start=False, stop=True)
                                pt = ptp.tile([128, 512], BF16, tag="pt")
                                nc.scalar.activation(
                                    pt[:, qs:], s_ps[:, qs:],
                                    mybir.ActivationFunctionType.Exp,
                                    scale=SCALE)
                                pts[jt] = pt

                            def consume4(j0, vh=vh, l_ps=l_ps, o_ps=o_ps,
                                         pts=pts, qss=qss, nj=nj):
                                # same-chain runs back-to-back: the PE
                                # streams chained same-bank matmuls at full
                                # rate; alternating banks costs ~150ns each
                                for jt in range(j0, j0 + 4):
                                    nc.tensor.matmul(
                                        l_ps[:, qss[jt]:], ones_b[:],
                                        pts[jt][:, qss[jt]:],
                                        start=(jt == 0), stop=(jt == nj - 1))
                                for jt in range(j0, j0 + 4):
                                    nc.tensor.matmul(
                                        o_ps[:, qss[jt]:], vh[:, jt],
                                        pts[jt][:, qss[jt]:],
                                        start=(jt == 0), stop=(jt == nj - 1))

                            # quad-wise software pipeline, 2-quad lookahead
                            for g in range(0, nj, 4):
                                for jj_ in range(4):
                                    emit_s(g + jj_)
                                if g >= 8:
                                    consume4(g - 8)
                            for g in sorted({max(0, nj - 8), nj - 4}):
                                consume4(g)

                            # 1/l: bf16 reciprocal on DVE
                            l_sb = bscp.tile([128, 512], BF16, tag="lsb")
                            nc.vector.tensor_copy(l_sb[:], l_ps[:])
                            r_sb = bscp.tile([128, 512], BF16, tag="r")
                            with nc.allow_low_precision(
                                    "softmax denom recip in bf16"):
                                nc.vector.reciprocal(r_sb[:], l_sb[:])
                            nc.vector.tensor_mul(
                                o_sb[:, h, isl], o_ps[:], r_sb[:])
                        if h + 2 < HC:
                            vhs.append(load_vh(h + 2))
                        else:
                            vhs.append(None)

                # ------------- Phase C: out projection -------------
                with tc.tile_pool(name="yo", bufs=4) as yop, \
                     tc.tile_pool(name="psC", bufs=4, space="PSUM") as pscp:
                    for tb in range(4):
                        ts = bass.ts(tb, 512)
                        for ct in range(16):
                            ps = pscp.tile([128, 512], F32, tag="ps")
                            for ht in range(8):
                                nc.tensor.matmul(
                                    ps[:], wps[:, ht, bass.ts(ct, 128)],
                                    o_sb[:, ht, ts],
                                    start=(ht == 0), stop=(ht == 7))
                            yo = yop.tile([128, 512], BF16, tag="yo")
                            nc.scalar.activation(
                                yo[:], ps[:],
                                mybir.ActivationFunctionType.Copy)
                            nc.sync.dma_start(
                                yT[ct * 128:(ct + 1) * 128, ts], yo[:])

    nc.compile()
    return nc


def _prep_inputs(x, freqs_cos, freqs_sin, W_attn, b_attn, W_proj):
    """Host-side sharding / layout prep.  Returns list of 8 in_maps."""
    perm = np.concatenate([np.arange(0, HD, 2), np.arange(1, HD, 2)])

    cosT = np.ascontiguousarray(freqs_cos.T)  # [64, T]
    sinT = np.ascontiguousarray(freqs_sin.T)
    rtab_u = np.concatenate([cosT, cosT], axis=0).astype(BF16_NP)
    rtab_v = np.concatenate([-sinT, sinT], axis=0).astype(BF16_NP)
    cos4 = np.tile(freqs_cos, (1, 4)).astype(BF16_NP)  # [T, 256]
    sin4 = np.tile(freqs_sin, (1, 4)).astype(BF16_NP)

    jj = np.arange(128)[:, None]
    ii = np.arange(128)[None, :]
    lneg = np.where(ii < jj, -1e9, 0.0).astype(BF16_NP)  # [128, 128]
    ident = np.eye(128).astype(BF16_NP)

    in_maps = []
    for core in range(NCORES):
        b = core // 2
        hs = HC * (core % 2)
        cols = np.concatenate(
            [g * HD + perm for g in range(hs, hs + HC)])  # [1024]

        wqk = np.concatenate(
            [W_attn[:, cols], W_attn[:, C + cols]], axis=1)
        bqk_flat = np.concatenate([b_attn[cols], b_attn[C + cols]])
        bqk = np.ascontiguousarray(
            bqk_flat.reshape(16, 128).T)  # [128, 16], bias[jt*128+p]
        wv = W_attn[:, 2 * C + cols]
        bv = np.broadcast_to(b_attn[2 * C + cols], (128, 1024))
        wp = W_proj[cols, :]

        in_maps.append({
            "xT": np.ascontiguousarray(x[b].T).astype(BF16_NP),
            "wqk": np.ascontiguousarray(wqk).astype(BF16_NP),
            "bqk": np.ascontiguousarray(bqk).astype(np.float32),
            "wv": np.ascontiguousarray(wv).astype(BF16_NP),
            "bv": np.ascontiguousarray(bv).astype(np.float32),
            "wp": np.ascontiguousarray(wp).astype(BF16_NP),
            "rtab_u": rtab_u,
            "rtab_v": rtab_v,
            "cos4": cos4,
            "sin4": sin4,
            "lneg": np.ascontiguousarray(lneg),
            "ident": np.ascontiguousarray(ident),
        })
    return in_maps


def kernel(x, freqs_cos, freqs_sin, mask, W_attn, b_attn, W_proj, b_proj,
           _return_results=False):
    x = np.asarray(x, dtype=np.float32)
    freqs_cos = np.asarray(freqs_cos, dtype=np.float32)
    freqs_sin = np.asarray(freqs_sin, dtype=np.float32)
    W_attn = np.asarray(W_attn, dtype=np.float32)
    b_attn = np.asarray(b_attn, dtype=np.float32)
    W_proj = np.asarray(W_proj, dtype=np.float32)
    b_proj = np.asarray(b_proj, dtype=np.float32)

    if "nc" not in _CACHE:
        _CACHE["nc"] = _build_nc()
    nc = _CACHE["nc"]

    in_maps = _prep_inputs(x, freqs_cos, freqs_sin, W_attn, b_attn, W_proj)
    res = run_bass_kernel_spmd(nc, in_maps, core_ids=list(range(NCORES)))

    out = np.empty((B, T, C), dtype=np.float32)
    for b in range(B):
        yt0 = res.results[2 * b]["yT"].astype(np.float32)
        yt1 = res.results[2 * b + 1]["yT"].astype(np.float32)
        out[b] = yt0.T + yt1.T + b_proj[None, :]
    if _return_results:
        return out, res
    return out
